# revision 46
# baseline (speedup 1.0000x reference)
"""HTSubTree forward as a distributed Bass kernel on 8 TRN2 NeuronCores.

out[b,u,v,r] = sum_{i,j,p} x[b,(i,j)] * WL[i,u,p] * WR2[j,v,p,r]
  where WL = f0*f1*c_left (left leaf pair + core) and
        WR2 = f2*f3*c_right*c_root, both precontracted on host (tiny).
Pure batch data-parallelism: 64 of 512 batch elements per core.

Per "g" step (2 pairs = 4 batch elements, 16 steps/core), all in bf16
except the fp32 PSUM accumulators (error ~0.4% << 2e-2 budget; output
is bf16, converted to fp32 on host):
  stage1 (x2, concurrent PE row-tiles via base partition q*64):
      psum_y_q[(b2,j), (par,c,u)] — one single-bank PSUM tile per pair
  relayout (x4 copies per pair): y2_q[(par,j), (c,b2,u)] bf16
  stage2 (x4 accum per pair, K=128=(par,j)): psum_o_q[(b2,u), (v,r)]
  evac: ot bf16 [128,512]; one 128KB out DMA per pair.
Scheduling constraints learned from traces:
  - pair q's relayout+evac run ENTIRELY on one engine (q=0 DVE,
    q=1 ACT): DVE and ACT may never touch the same PSUM bank
    concurrently, and the bank-aware tracker otherwise serializes the
    engines 1:1, doubling the critical path.
  - stage2 of step g-1 is emitted after step g's relayout so the PE's
    stage2 block overlaps the next relayout (the engine queues are
    strict FIFO).
  - steady state is vector-engine-bound at ~2.5-2.7us/step; the PE
    needs ~1.95us/step.
"""

import sys

sys.path.insert(0, "/opt/trn_rl_repo")

import numpy as np

import concourse.bass as bass
import concourse.tile as tile
from concourse import bacc, mybir
from concourse.bass_utils import run_bass_kernel_spmd

NCORES = 8
B = 512
BLOC = B // NCORES  # 64 batch elements per core
NTH = BLOC // 4     # 16 steps, 4 batch elements (2 pairs) each
F32 = mybir.dt.float32
F32R = mybir.dt.float32r
BF16 = mybir.dt.bfloat16

_COMPILED = None


def _build(reps=1):
    nc = bacc.Bacc("TRN2", target_bir_lowering=False, debug=False)
    # x: partitions (q,i), free (th, b2, j)
    x_ap = nc.dram_tensor("x", [128, NTH * 128], BF16, kind="ExternalInput").ap()
    # wlf2: WL duplicated on both partition halves; free = par*256 + c*64 + u
    wlf_ap = nc.dram_tensor("wlf", [128, 512], BF16, kind="ExternalInput").ap()
    # wr2c: [c][par*64+j][v*8+r] bf16
    wr2c_ap = nc.dram_tensor("wr2c", [4, 128, 512], BF16, kind="ExternalInput").ap()
    out_ap = nc.dram_tensor("out", [BLOC * 64, 512], BF16, kind="ExternalOutput").ap()

    with tile.TileContext(nc) as tc:
        with (
            tc.tile_pool(name="weights", bufs=1) as wpool,
            tc.tile_pool(name="xin", bufs=3) as xpool,
            tc.tile_pool(name="y2", bufs=4) as ypool,
            tc.tile_pool(name="ostage", bufs=4) as opool,
            tc.tile_pool(name="py", bufs=2, space="PSUM") as pypool,
            tc.tile_pool(name="po", bufs=2, space="PSUM") as popool,
        ):
          for _rep in range(reps):
            # prefetch the scalar-engine activation table during the boot
            # preamble (otherwise it lazily loads right before the first
            # relayout copy, ~1.3us on the critical path)
            scratch = wpool.tile([128, 128], BF16, tag="scratch")
            nc.gpsimd.memset(scratch[:], 0)
            nc.scalar.copy(scratch[:, 4:8], scratch[:, 0:4])

            # startup DMAs split between the scalar queue (free ~2us before
            # sync) and the sync queue, so stage1 inputs land ASAP.
            # xt0 goes first: it's 4x smaller than wlf, so stage1's inputs
            # complete earliest in this order.
            xt01 = []
            t = xpool.tile([128, 128], BF16, tag="xg", name="xt0")
            nc.scalar.dma_start(t[:], x_ap[:, 0:128])
            xt01.append(t)
            wlf = wpool.tile([128, 512], BF16, tag="wlf")
            nc.scalar.dma_start(wlf[:], wlf_ap[:])
            t = xpool.tile([128, 128], BF16, tag="xg", name="xt1")
            nc.scalar.dma_start(t[:], x_ap[:, 128:256])
            xt01.append(t)

            # HAM warmup: ~40 small dependency-free matmuls on the scratch
            # tile.  The scheduler runs them while the PE would otherwise
            # idle waiting for the input DMAs (~7-11.3us), so the PE's
            # activity window fills and HAM un-throttles the clock to
            # 2.4GHz BEFORE the real matmuls start (instead of at ~22us).
            warm = pypool.tile([128, 512], F32, tag="warm", space="PSUM",
                               bufs=1)
            for _w in range(40):
                nc.tensor.matmul(warm[0:8, 0:128], scratch[:, 0:8],
                                 scratch[:], start=True, stop=True)
            wr2 = []
            for h in range(2):
                t = wpool.tile([128, 1024], BF16, tag=f"wr2c{h}")
                nc.sync.dma_start(
                    t.rearrange("p (c f) -> p c f", c=2, f=512),
                    wr2c_ap.rearrange("c p f -> p c f")[:, 2 * h:2 * h + 2],
                )
                wr2.append(t)
            wr2 = [wr2[0][:, 0:512], wr2[0][:, 512:1024],
                   wr2[1][:, 0:512], wr2[1][:, 512:1024]]

            # per-"g" step: two pairs (q=0,1), each with its OWN single-bank
            # PSUM tiles; pair q's relayout+evac run entirely on one engine
            # (q=0 -> DVE, q=1 -> ACT) so the two engines never touch the
            # same PSUM bank and run fully concurrently.  stage2 of step
            # g-1 is emitted AFTER this step's relayout so the PE's stage2
            # block overlaps the DVE/ACT relayout of the next step.
            state = None
            for g in range(NTH):
                if g < 2:
                    xt = xt01[g]
                else:
                    xt = xpool.tile([128, 128], BF16, tag="xg")
                    nc.sync.dma_start(xt[:], x_ap[:, g * 128:(g + 1) * 128])

                # stage1: two concurrent row-tiles (q=0 rows 0:64, q=1 rows 64:128)
                py = [pypool.tile([128, 512], F32, tag=f"py{q}", name=f"py{q}",
                                  space="PSUM")
                      for q in range(2)]
                for q in range(2):
                    nc.tensor.matmul(
                        py[q][:],
                        xt[q * 64:(q + 1) * 64, :],
                        wlf[q * 64:(q + 1) * 64, :],
                        start=True, stop=True,
                    )

                # fill the pipeline-ramp PE gap (stage2(0) can't start until
                # the first relayouts finish) so HAM doesn't re-throttle
                if g < 2:
                    for _w in range(16):
                        nc.tensor.matmul(warm[0:8, 0:128], scratch[:, 0:8],
                                         scratch[:], start=True, stop=True)

                y2g = []
                for q in range(2):
                    # relayout: py[(b2,j), (par,c,u)] -> y2[(par,j), (c,b2,u)]
                    y2 = ypool.tile([128, 512], BF16, tag=f"y2{q}", name=f"y2{q}")
                    src_v = py[q].rearrange("(b2 j) (par c u) -> b2 par j c u",
                                            b2=2, j=64, par=2, c=4, u=64)
                    dst_v = y2.rearrange("(par j) (c b2 u) -> b2 par j c u",
                                         par=2, j=64, c=4, b2=2, u=64)
                    eng = nc.vector.tensor_copy if q == 0 else nc.scalar.copy
                    for b2 in range(2):
                        for par in range(2):
                            eng(dst_v[b2][par], src_v[b2][par])
                    y2g.append(y2)

                if state is not None:
                    _stage2_pair(nc, wr2, out_ap, popool, opool, *state)
                state = (y2g, g)

            _stage2_pair(nc, wr2, out_ap, popool, opool, *state)

    nc.compile()
    return nc


def _stage2_pair(nc, wr2, out_ap, popool, opool, y2g, g):
    pos, ots = [], []
    for q in range(2):
        # K=128 accumulating matmuls: lhsT = y2[:, c*128 : (c+1)*128]
        # po1 runs single-buffered to free a PSUM bank for the HAM warmup
        # tile; evac(g) finishes ~3.9us before stage2(g+1) needs the bank
        po = popool.tile([128, 512], mybir.dt.float32, tag=f"po{q}",
                         name=f"po{q}", space="PSUM", bufs=1 if q == 1 else 2)
        for c in range(4):
            nc.tensor.matmul(
                po[:],
                y2g[q][:, c * 128:(c + 1) * 128],
                wr2[c],
                start=(c == 0), stop=(c == 3),
            )
        pos.append(po)
    for q in range(2):
        t = 2 * g + q
        ot = opool.tile([128, 512], BF16, tag=f"ot{q}", name=f"ot{q}")
        eng = nc.vector.tensor_copy if q == 0 else nc.scalar.copy
        eng(ot[:], pos[q][:])
        # last step's pair-1 out-DMA rides the otherwise-idle scalar queue
        # so the two final DMAs run on separate queues in parallel
        deng = nc.scalar if (g == NTH - 1 and q == 1) else nc.sync
        deng.dma_start(out_ap[128 * t: 128 * (t + 1), :], ot[:])


def _host_prep(x, factors, cores):
    """Pre-contract the tiny parameters and lay out per-core shards."""
    f0, f1, f2, f3 = factors[0], factors[1], factors[2], factors[3]
    c_root, c_left, c_right = cores[0], cores[1], cores[2]
    # WL[(i0,i1),(o0,o1),p=r02]
    wl = np.einsum("ioa,jpb,abr->ijopr", f0, f1, c_left, optimize=True)
    wl = wl.reshape(64, 64, 8)  # [i, u, p]
    # WRq[(i2,i3),(o2,o3),q=r24];  WR2[j,v,p,r] = sum_q WRq * c_root[p,q,r]
    wrq = np.einsum("ioc,jpd,cdq->ijopq", f2, f3, c_right, optimize=True).reshape(64, 64, 8)
    wr2 = np.einsum("jvq,pqr->jvpr", wrq, c_root, optimize=True)  # [j, v, p, r]

    import ml_dtypes
    # wlf [64, 512]: free = par*256 + c*64 + u  with  p = 2c + par; dup rows
    wlf1 = np.ascontiguousarray(
        wl.reshape(64, 64, 4, 2).transpose(0, 3, 2, 1).reshape(64, 512))
    wlf = np.concatenate([wlf1, wlf1], axis=0).astype(ml_dtypes.bfloat16)
    # wr2c [4, 128, 512]: [c][par*64+j][v*8+r] = wr2[j, v, 2c+par, r]
    wr2c = np.ascontiguousarray(
        wr2.transpose(2, 0, 1, 3).reshape(4, 2, 64, 64, 8).reshape(4, 128, 512)
    ).astype(ml_dtypes.bfloat16)

    xf = x.reshape(B, 64, 64).astype(ml_dtypes.bfloat16)
    xs = []
    for core in range(NCORES):
        xl = xf[core * BLOC:(core + 1) * BLOC]  # [64(b), 64(i), 64(j)]
        # [th, q, b2, i, j] -> [q, i, th, b2, j]
        xr = xl.reshape(NTH, 2, 2, 64, 64).transpose(1, 3, 0, 2, 4)
        xs.append(np.ascontiguousarray(xr.reshape(128, NTH * 128)))
    return xs, wlf, wr2c


def kernel(x, factors, cores, _want_profile=False):
    global _COMPILED
    x = np.asarray(x, dtype=np.float32)
    factors = np.asarray(factors, dtype=np.float32)
    cores = np.asarray(cores, dtype=np.float32)
    if _COMPILED is None:
        _COMPILED = _build()
    nc = _COMPILED
    xs, wlf, wr2c = _host_prep(x, factors, cores)
    in_maps = [{"x": xs[c], "wlf": wlf, "wr2c": wr2c} for c in range(NCORES)]
    res = run_bass_kernel_spmd(nc, in_maps, list(range(NCORES)), trace=_want_profile)
    out = np.concatenate(
        [res.results[c]["out"].astype(np.float32).reshape(BLOC, 8, 8, 8, 8, 8)
         for c in range(NCORES)]
    )
    if _want_profile:
        return out, res
    return out


# revision 47
# speedup vs baseline: 1.0220x; 1.0220x over previous
"""HTSubTree forward as a distributed Bass kernel on 8 TRN2 NeuronCores.

out[b,u,v,r] = sum_{i,j,p} x[b,(i,j)] * WL[i,u,p] * WR2[j,v,p,r]
  where WL = f0*f1*c_left (left leaf pair + core) and
        WR2 = f2*f3*c_right*c_root, both precontracted on host (tiny).
Pure batch data-parallelism: 64 of 512 batch elements per core.

Per "g" step (2 pairs = 4 batch elements, 16 steps/core), all in bf16
except the fp32 PSUM accumulators (error ~0.4% << 2e-2 budget; output
is bf16, converted to fp32 on host):
  stage1 (x2, concurrent PE row-tiles via base partition q*64):
      psum_y_q[(b2,j), (par,c,u)] — one single-bank PSUM tile per pair
  relayout (x4 copies per pair): y2_q[(par,j), (c,b2,u)] bf16
  stage2 (x4 accum per pair, K=128=(par,j)): psum_o_q[(b2,u), (v,r)]
  evac: ot bf16 [128,512]; one 128KB out DMA per pair.
Scheduling constraints learned from traces:
  - pair q's relayout+evac run ENTIRELY on one engine (q=0 DVE,
    q=1 ACT): DVE and ACT may never touch the same PSUM bank
    concurrently, and the bank-aware tracker otherwise serializes the
    engines 1:1, doubling the critical path.
  - stage2 of step g-1 is emitted after step g's relayout so the PE's
    stage2 block overlaps the next relayout (the engine queues are
    strict FIFO).
  - steady state is vector-engine-bound at ~2.5-2.7us/step; the PE
    needs ~1.95us/step.
"""

import sys

sys.path.insert(0, "/opt/trn_rl_repo")

import numpy as np

import concourse.bass as bass
import concourse.tile as tile
from concourse import bacc, mybir
from concourse.bass_utils import run_bass_kernel_spmd

NCORES = 8
B = 512
BLOC = B // NCORES  # 64 batch elements per core
NTH = BLOC // 4     # 16 steps, 4 batch elements (2 pairs) each
F32 = mybir.dt.float32
F32R = mybir.dt.float32r
BF16 = mybir.dt.bfloat16

_COMPILED = None


def _build(reps=1):
    nc = bacc.Bacc("TRN2", target_bir_lowering=False, debug=False)
    # x: partitions (q,i), free (th, b2, j)
    x_ap = nc.dram_tensor("x", [128, NTH * 128], BF16, kind="ExternalInput").ap()
    # wlf2: WL duplicated on both partition halves; free = par*256 + c*64 + u
    wlf_ap = nc.dram_tensor("wlf", [128, 512], BF16, kind="ExternalInput").ap()
    # wr2c: [c][par*64+j][v*8+r] bf16
    wr2c_ap = nc.dram_tensor("wr2c", [4, 128, 512], BF16, kind="ExternalInput").ap()
    out_ap = nc.dram_tensor("out", [BLOC * 64, 512], BF16, kind="ExternalOutput").ap()

    with tile.TileContext(nc) as tc:
        with (
            tc.tile_pool(name="weights", bufs=1) as wpool,
            tc.tile_pool(name="xin", bufs=3) as xpool,
            tc.tile_pool(name="y2", bufs=4) as ypool,
            tc.tile_pool(name="ostage", bufs=4) as opool,
            tc.tile_pool(name="py", bufs=2, space="PSUM") as pypool,
            tc.tile_pool(name="po", bufs=2, space="PSUM") as popool,
        ):
          for _rep in range(reps):
            # prefetch the scalar-engine activation table during the boot
            # preamble (otherwise it lazily loads right before the first
            # relayout copy, ~1.3us on the critical path)
            scratch = wpool.tile([128, 128], BF16, tag="scratch")
            nc.vector.memset(scratch[:], 0)
            nc.scalar.copy(scratch[:, 4:8], scratch[:, 0:4])

            # startup DMAs split between the scalar queue (free ~2us before
            # sync) and the sync queue, so stage1 inputs land ASAP.
            # xt0 goes first: it's 4x smaller than wlf, so stage1's inputs
            # complete earliest in this order.
            xt01 = []
            t = xpool.tile([128, 128], BF16, tag="xg", name="xt0")
            nc.scalar.dma_start(t[:], x_ap[:, 0:128])
            xt01.append(t)
            wlf = wpool.tile([128, 512], BF16, tag="wlf")
            nc.scalar.dma_start(wlf[:], wlf_ap[:])
            t = xpool.tile([128, 128], BF16, tag="xg", name="xt1")
            nc.scalar.dma_start(t[:], x_ap[:, 128:256])
            xt01.append(t)

            # HAM warmup: ~40 small dependency-free matmuls on the scratch
            # tile.  The scheduler runs them while the PE would otherwise
            # idle waiting for the input DMAs (~7-11.3us), so the PE's
            # activity window fills and HAM un-throttles the clock to
            # 2.4GHz BEFORE the real matmuls start (instead of at ~22us).
            warm = pypool.tile([128, 512], F32, tag="warm", space="PSUM",
                               bufs=1)
            for _w in range(40):
                nc.tensor.matmul(warm[0:8, 0:128], scratch[:, 0:8],
                                 scratch[:], start=True, stop=True)
            wr2 = []
            for h in range(2):
                t = wpool.tile([128, 1024], BF16, tag=f"wr2c{h}")
                nc.sync.dma_start(
                    t.rearrange("p (c f) -> p c f", c=2, f=512),
                    wr2c_ap.rearrange("c p f -> p c f")[:, 2 * h:2 * h + 2],
                )
                wr2.append(t)
            wr2 = [wr2[0][:, 0:512], wr2[0][:, 512:1024],
                   wr2[1][:, 0:512], wr2[1][:, 512:1024]]

            # per-"g" step: two pairs (q=0,1), each with its OWN single-bank
            # PSUM tiles; pair q's relayout+evac run entirely on one engine
            # (q=0 -> DVE, q=1 -> ACT) so the two engines never touch the
            # same PSUM bank and run fully concurrently.  stage2 of step
            # g-1 is emitted AFTER this step's relayout so the PE's stage2
            # block overlaps the DVE/ACT relayout of the next step.
            state = None
            for g in range(NTH):
                if g < 2:
                    xt = xt01[g]
                else:
                    xt = xpool.tile([128, 128], BF16, tag="xg")
                    nc.sync.dma_start(xt[:], x_ap[:, g * 128:(g + 1) * 128])

                # stage1: two concurrent row-tiles (q=0 rows 0:64, q=1 rows 64:128)
                py = [pypool.tile([128, 512], F32, tag=f"py{q}", name=f"py{q}",
                                  space="PSUM")
                      for q in range(2)]
                for q in range(2):
                    nc.tensor.matmul(
                        py[q][:],
                        xt[q * 64:(q + 1) * 64, :],
                        wlf[q * 64:(q + 1) * 64, :],
                        start=True, stop=True,
                    )

                # fill the pipeline-ramp PE gap (stage2(0) can't start until
                # the first relayouts finish) so HAM doesn't re-throttle
                if g < 2:
                    for _w in range(16):
                        nc.tensor.matmul(warm[0:8, 0:128], scratch[:, 0:8],
                                         scratch[:], start=True, stop=True)

                y2g = []
                for q in range(2):
                    # relayout: py[(b2,j), (par,c,u)] -> y2[(par,j), (c,b2,u)]
                    y2 = ypool.tile([128, 512], BF16, tag=f"y2{q}", name=f"y2{q}")
                    src_v = py[q].rearrange("(b2 j) (par c u) -> b2 par j c u",
                                            b2=2, j=64, par=2, c=4, u=64)
                    dst_v = y2.rearrange("(par j) (c b2 u) -> b2 par j c u",
                                         par=2, j=64, c=4, b2=2, u=64)
                    eng = nc.vector.tensor_copy if q == 0 else nc.scalar.copy
                    for b2 in range(2):
                        for par in range(2):
                            eng(dst_v[b2][par], src_v[b2][par])
                    y2g.append(y2)

                if state is not None:
                    _stage2_pair(nc, wr2, out_ap, popool, opool, *state)
                state = (y2g, g)

            _stage2_pair(nc, wr2, out_ap, popool, opool, *state)

    nc.compile()
    return nc


def _stage2_pair(nc, wr2, out_ap, popool, opool, y2g, g):
    pos, ots = [], []
    for q in range(2):
        # K=128 accumulating matmuls: lhsT = y2[:, c*128 : (c+1)*128]
        # po1 runs single-buffered to free a PSUM bank for the HAM warmup
        # tile; evac(g) finishes ~3.9us before stage2(g+1) needs the bank
        po = popool.tile([128, 512], mybir.dt.float32, tag=f"po{q}",
                         name=f"po{q}", space="PSUM", bufs=1 if q == 1 else 2)
        for c in range(4):
            nc.tensor.matmul(
                po[:],
                y2g[q][:, c * 128:(c + 1) * 128],
                wr2[c],
                start=(c == 0), stop=(c == 3),
            )
        pos.append(po)
    for q in range(2):
        t = 2 * g + q
        ot = opool.tile([128, 512], BF16, tag=f"ot{q}", name=f"ot{q}")
        eng = nc.vector.tensor_copy if q == 0 else nc.scalar.copy
        eng(ot[:], pos[q][:])
        # last step's pair-1 out-DMA rides the otherwise-idle scalar queue
        # so the two final DMAs run on separate queues in parallel
        deng = nc.scalar if (g == NTH - 1 and q == 1) else nc.sync
        deng.dma_start(out_ap[128 * t: 128 * (t + 1), :], ot[:])


def _host_prep(x, factors, cores):
    """Pre-contract the tiny parameters and lay out per-core shards."""
    f0, f1, f2, f3 = factors[0], factors[1], factors[2], factors[3]
    c_root, c_left, c_right = cores[0], cores[1], cores[2]
    # WL[(i0,i1),(o0,o1),p=r02]
    wl = np.einsum("ioa,jpb,abr->ijopr", f0, f1, c_left, optimize=True)
    wl = wl.reshape(64, 64, 8)  # [i, u, p]
    # WRq[(i2,i3),(o2,o3),q=r24];  WR2[j,v,p,r] = sum_q WRq * c_root[p,q,r]
    wrq = np.einsum("ioc,jpd,cdq->ijopq", f2, f3, c_right, optimize=True).reshape(64, 64, 8)
    wr2 = np.einsum("jvq,pqr->jvpr", wrq, c_root, optimize=True)  # [j, v, p, r]

    import ml_dtypes
    # wlf [64, 512]: free = par*256 + c*64 + u  with  p = 2c + par; dup rows
    wlf1 = np.ascontiguousarray(
        wl.reshape(64, 64, 4, 2).transpose(0, 3, 2, 1).reshape(64, 512))
    wlf = np.concatenate([wlf1, wlf1], axis=0).astype(ml_dtypes.bfloat16)
    # wr2c [4, 128, 512]: [c][par*64+j][v*8+r] = wr2[j, v, 2c+par, r]
    wr2c = np.ascontiguousarray(
        wr2.transpose(2, 0, 1, 3).reshape(4, 2, 64, 64, 8).reshape(4, 128, 512)
    ).astype(ml_dtypes.bfloat16)

    xf = x.reshape(B, 64, 64).astype(ml_dtypes.bfloat16)
    xs = []
    for core in range(NCORES):
        xl = xf[core * BLOC:(core + 1) * BLOC]  # [64(b), 64(i), 64(j)]
        # [th, q, b2, i, j] -> [q, i, th, b2, j]
        xr = xl.reshape(NTH, 2, 2, 64, 64).transpose(1, 3, 0, 2, 4)
        xs.append(np.ascontiguousarray(xr.reshape(128, NTH * 128)))
    return xs, wlf, wr2c


def kernel(x, factors, cores, _want_profile=False):
    global _COMPILED
    x = np.asarray(x, dtype=np.float32)
    factors = np.asarray(factors, dtype=np.float32)
    cores = np.asarray(cores, dtype=np.float32)
    if _COMPILED is None:
        _COMPILED = _build()
    nc = _COMPILED
    xs, wlf, wr2c = _host_prep(x, factors, cores)
    in_maps = [{"x": xs[c], "wlf": wlf, "wr2c": wr2c} for c in range(NCORES)]
    res = run_bass_kernel_spmd(nc, in_maps, list(range(NCORES)), trace=_want_profile)
    out = np.concatenate(
        [res.results[c]["out"].astype(np.float32).reshape(BLOC, 8, 8, 8, 8, 8)
         for c in range(NCORES)]
    )
    if _want_profile:
        return out, res
    return out
